# revision 16
# baseline (speedup 1.0000x reference)
"""2-layer GAT on 8 Trainium2 NeuronCores — v2 (fp16, no L1 AllGather).

Strategy: dst-shard nodes (6250/core, padded 6272). Layer 1: every core
computes the FULL node table h1ext = x @ [W1|Ws1|Wd1] locally (fp16, rows
rotated so own nodes come first), writes it to local DRAM, then processes its
own dst nodes in degree-sorted CSR tiles: dma_gather pulls 512B rows
[h1 (c,h)-interleaved | as1] per in-edge, attention runs on DVE in fp16
(2x_1p mode throughout: heads innermost), alpha-weighted sums via in-place
fold-halving. Layer 2: per-core shard of h2ext is AllGathered in two chunks
(overlapped with compute), then the same gather/attention machinery with
256B rows. int16 gather indices limit one gather to 32768 rows, so each
layer splits its table into two halves (lo/hi rotated regions for L1,
chunk a/b for L2) with two gathers per supertile.
"""

import numpy as np

N = 50000
E = 800000
R = 8
NPC = N // R  # 6250 owned nodes per core
TPC = 49  # tiles of 128 nodes
NPAD = TPC * 128  # 6272 rows per shard
HALF = 4 * NPAD  # 25088 rotated rows per L1 half
IN_CH = 128
HIDDEN = 32
HEADS = 4
OUT_CH = 64
NEG_SLOPE = 0.2
EL1 = 256  # L1 table row: [h1 (c,h) 128 | as1 4 | dead] fp16, 512B
EL2 = 128  # L2 table row: [h2 64 | as2 | dead] fp16, 256B
SENT1 = 6250  # L1 sentinel row (own pad / region-4 pad) in each half
R1 = 3200  # L2 chunk split: ppos < R1 -> chunk a (tiles 0..24)
R1P = R1 + 16  # chunk-a shard rows incl sentinel row at R1
NA = R * R1P  # AllGather'd chunk-a rows
NB = R * (NPAD - R1)  # 24576 chunk-b rows
SENT2B = NPC - R1  # 3050: core-0 pad row in chunk b
CAP = 40  # J*(Dl+Dh) <= CAP (gather buffer sizing)
NEG = -60000.0  # fp16-safe "-inf" for pad alpha_src
EPS = 2e-5  # den epsilon; 1/EPS must stay inside fp16 range
TILE_A = 12  # node tiles per phase-A chunk (4 PSUM banks of 3)


# ---------------------------------------------------------------- host planner
def _build_supers(dlo_t, dhi_t, breaks=(25,)):
    """Greedy supertiles: J in (4,2,1), J*(Dl+Dh) <= CAP, never across breaks.
    Merging J tiles pads each to the group max degree; accept a merge only if
    the added descriptor cost (~6.3ns each) is below the saved per-gather
    fixed cost (~2us per J-1 merged pair of gathers)."""
    supers = []
    t = 0
    while t < TPC:
        best = None
        for J in (4, 2, 1):
            if t + J > TPC:
                continue
            if any(t < b < t + J for b in breaks):
                continue
            Dl = int(max(dlo_t[t : t + J]))
            Dh = int(max(dhi_t[t : t + J]))
            if J * (Dl + Dh) > CAP and J > 1:
                continue
            added = sum(
                (Dl - int(dlo_t[i])) + (Dh - int(dhi_t[i])) for i in range(t, t + J)
            )
            if J > 1 and added * 128 * 6.3 > (J - 1) * 2 * 994.0:
                continue
            best = (t, J, max(Dl, 1), max(Dh, 1))
            break
        supers.append(best)
        t += best[1]
    return supers


def _wrap_cols(lpos_dst, half, rel, supers, sents):
    """Slot tables for one (core, layer): per supertile x half, a dense
    [J*128, D] table of source rows (k-major flat order), int16-wrapped."""
    key = lpos_dst * 2 + half
    order = np.argsort(key, kind="stable")
    key_s = key[order]
    rel_s = rel[order]
    first = np.searchsorted(key_s, key_s)
    rank = np.arange(len(key_s)) - first
    cols = []
    for t0, J, Dl, Dh in supers:
        n0 = t0 * 128
        n1 = (t0 + J) * 128
        S = n1 - n0
        for h, D, sent in ((0, Dl, sents[0]), (1, Dh, sents[1])):
            tab = np.full((S, D), sent, dtype=np.int64)
            sel = (key_s % 2 == h) & (key_s // 2 >= n0) & (key_s // 2 < n1)
            rr = rank[sel]
            assert (rr < D).all(), "rank exceeded tile max degree"
            tab[key_s[sel] // 2 - n0, rr] = rel_s[sel]
            flat = np.empty(S * D, dtype=np.int16)
            nodes = np.arange(S)
            j = nodes // 128
            p = nodes % 128
            for k in range(D):
                flat[(k * J + j) * 128 + p] = tab[nodes, k]
            wrapped = flat.reshape(-1, 16)
            w = np.empty((128, S * D // 16), dtype=np.int16)
            for q in range(8):
                w[q * 16 : (q + 1) * 16, :] = wrapped.T
            cols.append(w)
    return cols


def _build_plan(edge_index):
    src = np.concatenate([edge_index[0], np.arange(N, dtype=np.int64)])
    dst = np.concatenate([edge_index[1], np.arange(N, dtype=np.int64)])
    own_s = src // NPC  # owner core of each edge's source
    own_d = dst // NPC

    # per-owner-region in-degree counts
    cnt = np.zeros((R, N), dtype=np.int64)
    for r in range(R):
        cnt[r] = np.bincount(dst[own_s == r], minlength=N)
    deg = cnt.sum(axis=0)

    # per-core L1 lo-degree (rotated: lo = owners c..c+3)
    d1lo = np.zeros((R, N), dtype=np.int64)
    for c in range(R):
        for i in range(4):
            d1lo[c] += cnt[(c + i) % R]
    d1hi = deg[None, :] - d1lo

    # L2 chunk coloring: greedy discrepancy pass assigns each source to chunk
    # a or b (quota R1 / NPC-R1 per core) so every dst's in-edges split evenly
    order_e = np.argsort(src, kind="stable")
    s_sorted = src[order_e]
    d_sorted = dst[order_e]
    indptr = np.searchsorted(s_sorted, np.arange(N + 1))
    bal = np.zeros(N, dtype=np.int32)
    acolor = np.zeros(N, dtype=bool)
    ca = np.zeros(R, dtype=np.int64)
    cb = np.zeros(R, dtype=np.int64)
    rng = np.random.default_rng(0)
    for s in rng.permutation(N):
        nbrs = d_sorted[indptr[s] : indptr[s + 1]]
        c = s // NPC
        if ca[c] >= R1:
            a = False
        elif cb[c] >= NPC - R1:
            a = True
        else:
            a = bal[nbrs].sum() < 0
        acolor[s] = a
        if a:
            ca[c] += 1
            bal[nbrs] += 1
        else:
            cb[c] += 1
            bal[nbrs] -= 1

    # per-core permutation: a-colored nodes first (tiles 0..24), then
    # b-colored (tiles 25..48), each run sorted by L1 degree key desc
    d2a = np.bincount(dst[acolor[src]], minlength=N)
    d2b = deg - d2a
    perms = []
    ppos = np.empty(N, dtype=np.int64)
    for c in range(R):
        ids = np.arange(c * NPC, (c + 1) * NPC)
        key = (
            np.maximum.reduce([d1lo[c][ids], d1hi[c][ids], d2a[ids], d2b[ids]]) * 1000
            + deg[ids]
        )
        ids_a = ids[acolor[ids]]
        ids_b = ids[~acolor[ids]]
        pa = ids_a[np.argsort(-key[acolor[ids]], kind="stable")]
        pb = ids_b[np.argsort(-key[~acolor[ids]], kind="stable")]
        p = np.concatenate([pa, pb])
        perms.append(p)
        ppos[p] = np.arange(NPC)

    # shared per-tile max degrees
    def tile_max(dvals_percore):
        m = np.zeros(TPC, dtype=np.int64)
        for c in range(R):
            dv = dvals_percore(c)
            dv = np.concatenate([dv, np.zeros(NPAD - NPC, dtype=np.int64)])
            m = np.maximum(m, dv.reshape(TPC, 128).max(axis=1))
        return m

    dlo1_t = tile_max(lambda c: d1lo[c][perms[c]])
    dhi1_t = tile_max(lambda c: d1hi[c][perms[c]])
    dlo2_t = tile_max(lambda c: d2a[perms[c]])
    dhi2_t = tile_max(lambda c: d2b[perms[c]])

    supers1 = _build_supers(dlo1_t, dhi1_t, breaks=())
    supers2 = _build_supers(dlo2_t, dhi2_t, breaks=())

    slots1 = sum(128 * J * (Dl + Dh) for (_, J, Dl, Dh) in supers1)
    slots2 = sum(128 * J * (Dl + Dh) for (_, J, Dl, Dh) in supers2)
    real = E + N
    plan = {
        "supers1": supers1,
        "supers2": supers2,
        "perms": perms,
        "inflation1": slots1 * R / real,
        "inflation2": slots2 * R / real,
    }

    # per-core gather index tables
    gidx_cores = []
    for c in range(R):
        own = own_d == c
        s_own = src[own]
        d_own = dst[own]
        lpos = ppos[d_own]

        rot = ((own_s[own] - c) % R) * NPAD + ppos[s_own]
        half1 = (rot >= HALF).astype(np.int64)
        rel1 = rot - half1 * HALF
        cols1 = _wrap_cols(lpos, half1, rel1, supers1, (SENT1, SENT1))

        half2 = (ppos[s_own] >= R1).astype(np.int64)
        rel2 = np.where(
            half2 == 0,
            own_s[own] * R1P + ppos[s_own],
            own_s[own] * (NPAD - R1) + (ppos[s_own] - R1),
        )
        cols2 = _wrap_cols(lpos, half2, rel2, supers2, (R1, SENT2B))
        gidx_cores.append(np.concatenate(cols1 + cols2, axis=1))
    plan["gidx"] = gidx_cores
    plan["W"] = gidx_cores[0].shape[1]
    assert all(g.shape[1] == plan["W"] for g in gidx_cores)
    assert plan["W"] * 2 == (slots1 + slots2) // 8
    return plan


# ---------------------------------------------------------------- bass kernel
def _build_bass(plan, phases="FULL"):
    import concourse.bacc as bacc
    import concourse.mybir as mybir
    import concourse.tile as tile
    from concourse.masks import make_identity

    f32 = mybir.dt.float32
    f16 = mybir.dt.float16
    i16 = mybir.dt.int16
    AX = mybir.AxisListType.X
    OP = mybir.AluOpType
    AF = mybir.ActivationFunctionType

    supers1 = plan["supers1"]
    supers2 = plan["supers2"]
    W = plan["W"]
    NT = R * NPAD // 128  # 392 node tiles in the full sweep
    gmax1 = max(J * (Dl + Dh) for (_, J, Dl, Dh) in supers1)
    gmax2 = max(J * (Dl + Dh) for (_, J, Dl, Dh) in supers2)
    GW = max(gmax1 * EL1, gmax2 * EL2)  # gather tile elems (fp16)
    SW = max(gmax1 * 128, gmax2 * 64)  # scr tile elems
    EW = max(gmax1 * 4, gmax2)  # e tile elems

    nc = bacc.Bacc(
        "TRN2",
        target_bir_lowering=False,
        debug=False,
        num_devices=R,
        num_swdge_queues=4,
        dynamic_dma_scratch_size=32768,
    )
    xT_in = nc.dram_tensor("xT", [128, R * NPAD], f16, kind="ExternalInput")
    gidx_in = nc.dram_tensor("gidx", [128, W], i16, kind="ExternalInput")
    wcat1_in = nc.dram_tensor("wcat1", [128, 136], f16, kind="ExternalInput")
    wcat2_in = nc.dram_tensor("wcat2", [128, 66], f16, kind="ExternalInput")
    b1_in = nc.dram_tensor("b1c", [128, 1], f32, kind="ExternalInput")
    b2_in = nc.dram_tensor("b2r", [1, 64], f32, kind="ExternalInput")
    cc_in = nc.dram_tensor("ccn", [128, 1], f32, kind="ExternalInput")
    out_d = nc.dram_tensor("out", [NPAD, 64], f32, kind="ExternalOutput")

    qrr = [0]  # SWDGE queue round-robin

    with tile.TileContext(nc) as tc:
        with (
            tc.tile_pool(name="const", bufs=1) as cp,
            tc.tile_pool(name="xt", bufs=3) as xp,
            tc.tile_pool(name="stage", bufs=2) as sp,
            tc.tile_pool(name="gath", bufs=3) as gp,
            tc.tile_pool(name="scrp", bufs=2) as scp,
            tc.tile_pool(name="work", bufs=2) as wp,
            tc.tile_pool(name="psA", bufs=4, space="PSUM") as ppa,
            tc.tile_pool(name="psT", bufs=2, space="PSUM") as ppt,
            tc.tile_pool(name="ps2", bufs=2, space="PSUM") as pp2,
            tc.tile_pool(name="dram", bufs=1, space="DRAM") as dp,
        ):
            table1 = dp.tile([R * NPAD, EL1], f16)
            shard2a = dp.tile([R1P, EL2], f16)
            shard2b = dp.tile([NPAD - R1, EL2], f16)
            table2a = dp.tile([NA, EL2], f16, addr_space="Shared")
            table2b = dp.tile([NB, EL2], f16, addr_space="Shared")

            wcat1 = cp.tile([128, 136], f16)
            nc.sync.dma_start(out=wcat1[:], in_=wcat1_in[:])
            wcat2 = cp.tile([128, 66], f16)
            nc.sync.dma_start(out=wcat2[:], in_=wcat2_in[:])
            b1c = cp.tile([128, 1], f32)
            nc.sync.dma_start(out=b1c[:], in_=b1_in[:])
            ccn = cp.tile([128, 1], f32)
            nc.sync.dma_start(out=ccn[:], in_=cc_in[:])
            b2p = cp.tile([1, 64], f32)
            nc.sync.dma_start(out=b2p[:1, :], in_=b2_in[:])
            b2b = cp.tile([128, 64], f32)
            nc.gpsimd.partition_broadcast(b2b[:], b2p[:1, :])
            ident = cp.tile([128, 128], f16)
            make_identity(nc, ident[:])
            idxall = cp.tile([128, W], i16)
            nc.sync.dma_start(out=idxall[:], in_=gidx_in[:])
            adbuf1 = cp.tile([128, 4 * TPC], f16)
            adbuf2 = cp.tile([128, TPC], f16)
            epsc = cp.tile([128, 16], f32)
            nc.gpsimd.memset(epsc[:], EPS)
            neg = cp.tile([128, 4], f16)
            nc.gpsimd.memset(neg[:], NEG)
            sent2 = cp.tile([128, EL2], f16)
            nc.gpsimd.memset(sent2[:], 0.0)
            nc.gpsimd.memset(sent2[:, 64:65], NEG)
            nc.sync.dma_start(out=shard2a[R1 : R1 + 16, :], in_=sent2[0:16, :])

            # ---------------- phase A: full table1 sweep (rotated rows)
            t = 0
            while t < NT:
                nt = min(TILE_A, NT - t)
                xt = xp.tile([128, TILE_A * 128], f16, tag="xt")
                nc.sync.dma_start(
                    out=xt[:, 0 : nt * 128], in_=xT_in[:, t * 128 : (t + nt) * 128]
                )
                stg = sp.tile([128, TILE_A * EL1], f16, tag="stg")
                nc.gpsimd.memset(
                    stg[:].rearrange("p (j v) -> p j v", v=EL1)[:, 0:nt, 132:EL1], 0.0
                )
                for b0 in range(0, nt, 3):
                    nb = min(3, nt - b0)
                    psA = ppa.tile([128, 512], f32, tag="psA")
                    for jj in range(nb):
                        nc.tensor.matmul(
                            psA[:, jj * 136 : (jj + 1) * 136],
                            lhsT=xt[:, (b0 + jj) * 128 : (b0 + jj + 1) * 128],
                            rhs=wcat1[:],
                            start=True,
                            stop=True,
                        )
                    nc.vector.tensor_copy(
                        out=stg[:].rearrange("p (j v) -> p j v", v=EL1)[
                            :, b0 : b0 + nb, 0:132
                        ],
                        in_=psA[:, 0 : nb * 136].rearrange("p (j v) -> p j v", v=136)[
                            :, :, 0:132
                        ],
                    )
                    tb = t + b0
                    if tb < TPC:  # own tiles: extract alpha_dst columns
                        nn = min(nb, TPC - tb)
                        nc.vector.tensor_copy(
                            out=adbuf1[:, 4 * tb : 4 * (tb + nn)].rearrange(
                                "p (j v) -> p j v", v=4
                            ),
                            in_=psA[:, 0 : nn * 136].rearrange(
                                "p (j v) -> p j v", v=136
                            )[:, :, 132:136],
                        )
                eng = nc.sync if (t // TILE_A) % 2 == 0 else nc.scalar
                eng.dma_start(
                    out=table1[t * 128 : (t + nt) * 128, :].rearrange(
                        "(j p) v -> p j v", p=128
                    ),
                    in_=stg[:].rearrange("p (j v) -> p j v", v=EL1)[:, 0:nt],
                )
                t += nt

            # pad rows of every rotated region: alpha_src = NEG
            for r in range(R):
                nc.sync.dma_start(
                    out=table1[r * NPAD + NPC : (r + 1) * NPAD, 128:132],
                    in_=neg[0 : NPAD - NPC, :],
                )

            if phases == "A":
                for tt in range(TPC):
                    dbg16 = wp.tile([128, 64], f16, tag="dbg16")
                    nc.sync.dma_start(
                        out=dbg16[:],
                        in_=table1[tt * 128 : (tt + 1) * 128, 0:64].rearrange(
                            "(one p) v -> p (one v)", p=128
                        ),
                    )
                    dbg32 = wp.tile([128, 64], f32, tag="dbg32")
                    nc.vector.tensor_copy(out=dbg32[:], in_=dbg16[:])
                    nc.sync.dma_start(
                        out=out_d[tt * 128 : (tt + 1) * 128, :].rearrange(
                            "(one p) v -> p (one v)", p=128
                        ),
                        in_=dbg32[:],
                    )

            # ---------------- phase B/C shared attention machinery
            col = [0]

            def gather_pair(table_lo, table_hi, EL, t0, J, Dl, Dh):
                g = gp.tile([128, GW], f16, tag="g")
                for D, tab, off in ((Dl, table_lo, 0), (Dh, table_hi, J * Dl * EL)):
                    NI = 128 * J * D
                    nc.gpsimd.dma_gather(
                        g[:, off : off + J * D * EL].rearrange(
                            "p (c r) -> p c r", r=EL
                        ),
                        tab,
                        idxall[:, col[0] : col[0] + NI // 16],
                        NI,
                        NI,
                        EL,
                        single_packet=False,
                        queue_num=qrr[0] % 4,
                    )
                    qrr[0] += 1
                    col[0] += NI // 16
                return g

            def fold(scr, K, J, CW):
                """In-place fold of scr's kk axis (row stride J*CW) down to 1."""
                while K > 1:
                    K2 = (K + 1) // 2
                    nf = K - K2
                    nc.vector.tensor_tensor(
                        out=scr[:, 0 : nf * J * CW],
                        in0=scr[:, 0 : nf * J * CW],
                        in1=scr[:, K2 * J * CW : K * J * CW],
                        op=OP.add,
                    )
                    K = K2

            # ---------------- phase B: layer-1 attention + ELU + W2
            ag1_done = False
            for t0, J, Dl, Dh in supers1 if phases != "A" else []:
                K = Dl + Dh
                g = gather_pair(table1[0:HALF, :], table1[HALF : 2 * HALF, :], EL1,
                                t0, J, Dl, Dh)
                e = wp.tile([128, EW], f16, tag="e")
                for D, goff, eoff in ((Dl, 0, 0), (Dh, J * Dl * EL1, Dl * J * 4)):
                    gv = g[:, goff : goff + D * J * EL1].rearrange(
                        "p (k j r) -> p k j r", j=J, r=EL1
                    )
                    nc.vector.tensor_tensor(
                        out=e[:, eoff : eoff + D * J * 4].rearrange(
                            "p (k j h) -> p k j h", j=J, h=4
                        ),
                        in0=gv[:, :, :, 128:132],
                        in1=adbuf1[:, 4 * t0 : 4 * (t0 + J)]
                        .rearrange("p (j h) -> p j h", h=4)
                        .unsqueeze(1)
                        .to_broadcast([128, D, J, 4]),
                        op=OP.add,
                    )
                esc = wp.tile([128, EW], f16, tag="esc")
                nc.vector.tensor_scalar_mul(
                    esc[:, 0 : K * J * 4], e[:, 0 : K * J * 4], NEG_SLOPE
                )
                nc.vector.tensor_tensor(
                    e[:, 0 : K * J * 4], e[:, 0 : K * J * 4], esc[:, 0 : K * J * 4],
                    op=OP.max,
                )
                nc.scalar.activation(e[:, 0 : K * J * 4], e[:, 0 : K * J * 4], AF.Exp)
                den = wp.tile([128, 16], f32, tag="den")
                nc.vector.reduce_sum(
                    out=den[:, 0 : 4 * J],
                    in_=e[:, 0 : K * J * 4].rearrange(
                        "p (k j h) -> p j h k", j=J, h=4
                    ),
                    axis=AX,
                )
                nc.vector.tensor_tensor(
                    den[:, 0 : 4 * J], den[:, 0 : 4 * J], epsc[:, 0 : 4 * J], op=OP.add
                )
                rden = wp.tile([128, 16], f32, tag="rden")
                nc.vector.reciprocal(rden[:, 0 : 4 * J], den[:, 0 : 4 * J])
                rden16 = wp.tile([128, 16], f16, tag="rden16")
                nc.vector.tensor_copy(out=rden16[:, 0 : 4 * J], in_=rden[:, 0 : 4 * J])

                scr = scp.tile([128, SW], f16, tag="scr")
                for D, goff, soff, eoff in (
                    (Dl, 0, 0, 0),
                    (Dh, J * Dl * EL1, Dl * J * 128, Dl * J * 4),
                ):
                    nc.vector.tensor_tensor(
                        out=scr[:, soff : soff + D * J * 128].rearrange(
                            "p (m c h) -> p m c h", c=32, h=4
                        ),
                        in0=g[:, goff : goff + D * J * EL1]
                        .rearrange("p (m r) -> p m r", r=EL1)[:, :, 0:128]
                        .rearrange("p m (c h) -> p m c h", h=4),
                        in1=e[:, eoff : eoff + D * J * 4]
                        .rearrange("p (m h) -> p m h", h=4)
                        .unsqueeze(2)
                        .to_broadcast([128, D * J, 32, 4]),
                        op=OP.mult,
                    )
                fold(scr, K, J, 128)
                out1 = wp.tile([128, 512], f16, tag="out1")
                nc.vector.tensor_tensor(
                    out=out1[:, 0 : J * 128].rearrange(
                        "p (j c h) -> p j c h", c=32, h=4
                    ),
                    in0=scr[:, 0 : J * 128].rearrange("p (j c h) -> p j c h", c=32, h=4),
                    in1=rden16[:, 0 : 4 * J]
                    .rearrange("p (j h) -> p j h", h=4)
                    .unsqueeze(2)
                    .to_broadcast([128, J, 32, 4]),
                    op=OP.mult,
                )
                if phases == "B":
                    dbg32 = wp.tile([128, 64], f32, tag="dbg32")
                    for j in range(J):
                        nc.vector.tensor_copy(
                            out=dbg32[:], in_=out1[:, j * 128 : j * 128 + 64]
                        )
                        nc.sync.dma_start(
                            out=out_d[(t0 + j) * 128 : (t0 + j + 1) * 128, :].rearrange(
                                "(one p) v -> p (one v)", p=128
                            ),
                            in_=dbg32[:],
                        )
                    continue
                psT = ppt.tile([128, 1024], f16, tag="psT")
                for j in range(J):
                    nc.tensor.transpose(
                        psT[:, j * 128 : (j + 1) * 128],
                        out1[:, j * 128 : (j + 1) * 128],
                        ident[:],
                    )
                zt = wp.tile([128, 512], f16, tag="zt")
                nc.vector.tensor_scalar(
                    out=zt[:, 0 : J * 128], in0=psT[:, 0 : J * 128],
                    scalar1=b1c[:, 0:1], scalar2=None, op0=OP.add,
                )
                znt = wp.tile([128, 512], f16, tag="znt")
                nc.vector.tensor_scalar(
                    out=znt[:, 0 : J * 128], in0=zt[:, 0 : J * 128],
                    scalar1=0.0, scalar2=None, op0=OP.min,
                )
                emt = wp.tile([128, 512], f16, tag="emt")
                nc.scalar.activation(emt[:, 0 : J * 128], znt[:, 0 : J * 128], AF.Exp)
                rt = wp.tile([128, 512], f16, tag="rt")
                nc.vector.tensor_scalar(
                    out=rt[:, 0 : J * 128], in0=zt[:, 0 : J * 128],
                    scalar1=0.0, scalar2=None, op0=OP.max,
                )
                elut = wp.tile([128, 512], f16, tag="elut")
                nc.vector.tensor_tensor(
                    elut[:, 0 : J * 128], rt[:, 0 : J * 128], emt[:, 0 : J * 128],
                    op=OP.add,
                )
                ps2 = pp2.tile([128, 512], f32, tag="ps2")
                for j in range(J):
                    nc.tensor.matmul(
                        ps2[:, j * 66 : (j + 1) * 66],
                        lhsT=elut[:, j * 128 : (j + 1) * 128],
                        rhs=wcat2[:],
                        start=True,
                        stop=True,
                    )
                h2e = wp.tile([128, 264], f16, tag="h2e")
                nc.vector.tensor_copy(out=h2e[:, 0 : J * 66], in_=ps2[:, 0 : J * 66])
                nc.vector.tensor_scalar(
                    out=adbuf2[:, t0 : t0 + J].rearrange("p (j one) -> p j one", one=1),
                    in0=ps2[:, 0 : J * 66].rearrange("p (j v) -> p j v", v=66)[
                        :, :, 65:66
                    ],
                    scalar1=ccn[:, 0:1],
                    scalar2=None,
                    op0=OP.add,
                )
                # store h2|as2 rows into the right shard chunk
                for j in range(J):
                    tt = t0 + j
                    if tt < 25:
                        dstrows = shard2a[tt * 128 : (tt + 1) * 128, 0:65]
                    else:
                        dstrows = shard2b[(tt - 25) * 128 : (tt - 24) * 128, 0:65]
                    nc.sync.dma_start(
                        out=dstrows,
                        in_=h2e[:, j * 66 : j * 66 + 65],
                    )
                if phases == "FULL" and not ag1_done and t0 + J >= 25:
                    # chunk-a shard complete: fix own pads' as2? (pads are in b)
                    nc.gpsimd.collective_compute(
                        "AllGather",
                        mybir.AluOpType.bypass,
                        replica_groups=[list(range(R))],
                        ins=[shard2a.opt()],
                        outs=[table2a[:].opt()],
                    )
                    ag1_done = True

            # pad rows of own chunk-b shard: as2 = NEG, then AllGather
            if phases in ("A", "B"):
                supers2_eff = []
            else:
                supers2_eff = supers2
            if phases == "FULL":
                nc.sync.dma_start(
                    out=shard2b[SENT2B : NPAD - R1, 64:65],
                    in_=neg[0 : NPAD - R1 - SENT2B, 0:1],
                )
                nc.gpsimd.collective_compute(
                "AllGather",
                mybir.AluOpType.bypass,
                    replica_groups=[list(range(R))],
                    ins=[shard2b.opt()],
                    outs=[table2b[:].opt()],
                )

            # ---------------- phase C: layer-2 attention
            for t0, J, Dl, Dh in supers2_eff:
                K = Dl + Dh
                g = gather_pair(table2a[:], table2b[:], EL2, t0, J, Dl, Dh)
                e = wp.tile([128, EW], f16, tag="e")
                for D, goff, eoff in ((Dl, 0, 0), (Dh, J * Dl * EL2, Dl * J)):
                    gv = g[:, goff : goff + D * J * EL2].rearrange(
                        "p (k j r) -> p k j r", j=J, r=EL2
                    )
                    nc.vector.tensor_tensor(
                        out=e[:, eoff : eoff + D * J].rearrange(
                            "p (k j one) -> p k j one", j=J, one=1
                        ),
                        in0=gv[:, :, :, 64:65],
                        in1=adbuf2[:, t0 : t0 + J]
                        .unsqueeze(1)
                        .unsqueeze(3)
                        .to_broadcast([128, D, J, 1]),
                        op=OP.add,
                    )
                esc = wp.tile([128, EW], f16, tag="esc")
                nc.vector.tensor_scalar_mul(
                    esc[:, 0 : K * J], e[:, 0 : K * J], NEG_SLOPE
                )
                nc.vector.tensor_tensor(
                    e[:, 0 : K * J], e[:, 0 : K * J], esc[:, 0 : K * J], op=OP.max
                )
                nc.scalar.activation(e[:, 0 : K * J], e[:, 0 : K * J], AF.Exp)
                den = wp.tile([128, 16], f32, tag="den")
                nc.vector.reduce_sum(
                    out=den[:, 0:J],
                    in_=e[:, 0 : K * J].rearrange("p (k j) -> p j k", j=J),
                    axis=AX,
                )
                nc.vector.tensor_tensor(
                    den[:, 0:J], den[:, 0:J], epsc[:, 0:J], op=OP.add
                )
                rden = wp.tile([128, 16], f32, tag="rden")
                nc.vector.reciprocal(rden[:, 0:J], den[:, 0:J])
                rd2 = wp.tile([128, 8], f16, tag="rd2")
                nc.vector.tensor_copy(
                    out=rd2[:, 0 : 2 * J].rearrange("p (j two) -> p j two", two=2),
                    in_=rden[:, 0:J].unsqueeze(2).to_broadcast([128, J, 2]),
                )
                ex2 = wp.tile([128, 2 * max(gmax2, 1)], f16, tag="ex2")
                nc.vector.tensor_copy(
                    out=ex2[:, 0 : 2 * K * J].rearrange("p (m two) -> p m two", two=2),
                    in_=e[:, 0 : K * J].unsqueeze(2).to_broadcast([128, K * J, 2]),
                )
                scr = scp.tile([128, SW], f16, tag="scr")
                for D, goff, soff, eoff in (
                    (Dl, 0, 0, 0),
                    (Dh, J * Dl * EL2, Dl * J * 64, 2 * Dl * J),
                ):
                    nc.vector.tensor_tensor(
                        out=scr[:, soff : soff + D * J * 64].rearrange(
                            "p (m c two) -> p m c two", c=32, two=2
                        ),
                        in0=g[:, goff : goff + D * J * EL2]
                        .rearrange("p (m r) -> p m r", r=EL2)[:, :, 0:64]
                        .rearrange("p m (c two) -> p m c two", two=2),
                        in1=ex2[:, eoff : eoff + 2 * D * J]
                        .rearrange("p (m two) -> p m two", two=2)
                        .unsqueeze(2)
                        .to_broadcast([128, D * J, 32, 2]),
                        op=OP.mult,
                    )
                fold(scr, K, J, 64)
                out2 = wp.tile([128, 256], f16, tag="out2")
                nc.vector.tensor_tensor(
                    out=out2[:, 0 : J * 64].rearrange(
                        "p (j c two) -> p j c two", c=32, two=2
                    ),
                    in0=scr[:, 0 : J * 64].rearrange(
                        "p (j c two) -> p j c two", c=32, two=2
                    ),
                    in1=rd2[:, 0 : 2 * J]
                    .rearrange("p (j two) -> p j two", two=2)
                    .unsqueeze(2)
                    .to_broadcast([128, J, 32, 2]),
                    op=OP.mult,
                )
                out2f = wp.tile([128, 256], f32, tag="out2f")
                nc.vector.tensor_tensor(
                    out=out2f[:, 0 : J * 64].rearrange("p (j c) -> p j c", c=64),
                    in0=out2[:, 0 : J * 64].rearrange("p (j c) -> p j c", c=64),
                    in1=b2b[:].unsqueeze(1).to_broadcast([128, J, 64]),
                    op=OP.add,
                )
                nc.sync.dma_start(
                    out=out_d[t0 * 128 : (t0 + J) * 128, :].rearrange(
                        "(j p) v -> p j v", p=128
                    ),
                    in_=out2f[:, 0 : J * 64].rearrange("p (j v) -> p j v", v=64),
                )

    nc.finalize()
    return nc


# ---------------------------------------------------------------- entry point
_cache = {}

PCH = np.empty(128, dtype=np.int64)  # (c,h)-interleaved channel order
for _c in range(HIDDEN):
    for _h in range(HEADS):
        PCH[_c * HEADS + _h] = _h * HIDDEN + _c


def kernel(x, edge_index, W1, att_src1, att_dst1, b1, W2, att_src2, att_dst2, b2):
    from concourse.bass_utils import run_bass_kernel_spmd

    x = np.asarray(x, dtype=np.float32)
    edge_index = np.asarray(edge_index, dtype=np.int64)
    W1 = np.asarray(W1, dtype=np.float32)
    W2 = np.asarray(W2, dtype=np.float32)
    att_src1 = np.asarray(att_src1, dtype=np.float32)
    att_dst1 = np.asarray(att_dst1, dtype=np.float32)
    att_src2 = np.asarray(att_src2, dtype=np.float32)
    att_dst2 = np.asarray(att_dst2, dtype=np.float32)
    b1 = np.asarray(b1, dtype=np.float32)
    b2 = np.asarray(b2, dtype=np.float32)

    import os

    phases = os.environ.get("KERNEL_PHASES", "FULL")
    key = (hash(edge_index.tobytes()), phases)
    if _cache.get("key") != key:
        if _cache.get("plan_key") != key[0]:
            _cache["plan"] = _build_plan(edge_index)
            _cache["plan_key"] = key[0]
        _cache["nc"] = _build_bass(_cache["plan"], phases)
        _cache["key"] = key
    plan = _cache["plan"]
    nc = _cache["nc"]

    # weight packing: (c,h)-interleaved channels; as/ad folded into the matmul
    W1r = W1.reshape(IN_CH, HEADS, HIDDEN)
    Ws1 = np.einsum("khc,hc->kh", W1r, att_src1)
    Wd1 = np.einsum("khc,hc->kh", W1r, att_dst1)
    wcat1 = np.concatenate([W1[:, PCH], Ws1, Wd1], axis=1).astype(np.float16)
    Ws2 = W2 @ att_src2[0]
    Wd2 = W2 @ att_dst2[0]
    wcat2 = np.concatenate(
        [W2[PCH, :], Ws2[PCH, None], Wd2[PCH, None]], axis=1
    ).astype(np.float16)
    S = W2.sum(axis=0)
    Cs = float(S @ att_src2[0])
    Cd = float(S @ att_dst2[0])
    b2p = (b2 - S).astype(np.float32).reshape(1, 64)
    ccn = np.full((128, 1), -(Cs + Cd), dtype=np.float32)
    b1c = b1[PCH].reshape(128, 1).astype(np.float32)

    x16T = np.ascontiguousarray(x.T.astype(np.float16))  # [128, N]
    in_maps = []
    for c in range(R):
        xT = np.zeros((128, R * NPAD), dtype=np.float16)
        for r in range(R):
            region = plan["perms"][(c + r) % R]
            xT[:, r * NPAD : r * NPAD + NPC] = x16T[:, region]
        in_maps.append(
            {
                "xT": xT,
                "gidx": plan["gidx"][c],
                "wcat1": wcat1,
                "wcat2": wcat2,
                "b1c": b1c,
                "b2r": b2p,
                "ccn": ccn,
            }
        )

    res = run_bass_kernel_spmd(nc, in_maps, core_ids=list(range(R)))
    _cache["last_res"] = res
    out = np.empty((N, OUT_CH), dtype=np.float32)
    for c in range(R):
        out[plan["perms"][c]] = res.results[c]["out"][:NPC]
    return out
